# revision 6
# baseline (speedup 1.0000x reference)
"""Adaptive-softmax hyperlink-prediction loss on 8 Trainium2 NeuronCores.

Strategy (vocab-parallel, per sharding hint): every core receives the full
T=2048 gathered embeddings plus a column shard of each classifier matrix
(head 10002(+pad)->1280/core, tail1 30000->3750/core, tail2 60000->7500/core).
Each core computes its shard's logits, exp's them (fused row-sum on the
scalar engine) and emits per-row partial sumexp; the target logit itself is
computed as a row-wise dot between the embedding and the *gathered* weight
row (host-gathered, so no core owns-the-column bookkeeping is needed), with
each core handling 256 tokens.  Host combines: LSE = log(sum of partials),
out = g_head - LSE_head + masked tail terms.

All matmul inputs are bf16 (fp32 accumulation in PSUM); logit abs error is
~2e-3 on |out| ~ 10-20, i.e. ~1e-4 relative.
"""

import os
import sys

import numpy as np

for _p in ("/opt/trn_rl_repo", "/root/.axon_site/_ro/trn_rl_repo"):
    if os.path.isdir(_p) and _p not in sys.path:
        sys.path.insert(0, _p)

import ml_dtypes  # noqa: E402

import concourse.bacc as bacc  # noqa: E402
import concourse.mybir as mybir  # noqa: E402
import concourse.tile as tile  # noqa: E402
from concourse.bass_utils import run_bass_kernel_spmd  # noqa: E402

BF16 = ml_dtypes.bfloat16

# Problem constants (hardcoded per contract)
HDIM = 768
IN_F = 2 * HDIM                  # 1536
CUTOFFS = [10000, 40000, 100000]
SHORTLIST = CUTOFFS[0]           # 10000
HEAD_SIZE = SHORTLIST + 2        # 10002
T1_DIM, T1_SIZE = IN_F // 4, CUTOFFS[1] - CUTOFFS[0]   # 384, 30000
T2_DIM, T2_SIZE = IN_F // 16, CUTOFFS[2] - CUTOFFS[1]  # 96, 60000
T = 2048
NCORES = 8
TPC = T // NCORES                # 256 tokens/core for the gathered dot

KA = IN_F + 128                  # 1664 = bias-augmented contraction (13*128)
KCH = KA // 128                  # 13 K-chunks (chunk 12 row 0 = ones/bias)
MCH = T // 128                   # 16 token chunks
HV = 1280                        # head vocab per core (10240 padded)
HVPAD = HV * NCORES              # 10240
V1 = T1_SIZE // NCORES           # 3750
V2 = T2_SIZE // NCORES           # 7500
PAD_BIAS = -87.0                 # exp(-87) ~ 1.6e-38: kills padded head cols

K1CH = T1_DIM // 128             # 3
PK2 = T2_DIM                     # 96 partitions for tail2 contraction


def _ntiles(n):
    """Split free dim n into <=512 matmul/PSUM tiles."""
    out = []
    off = 0
    while off < n:
        sz = min(512, n - off)
        out.append((off, sz))
        off += sz
    return out


HT = _ntiles(HV)    # 3 tiles
T1T = _ntiles(V1)   # 8 tiles
T2T = _ntiles(V2)   # 15 tiles

_CACHE = {}


def _build_program(level=5):
    if ("nc", level) in _CACHE:
        return _CACHE[("nc", level)]

    bf = mybir.dt.bfloat16
    f32 = mybir.dt.float32
    nc = bacc.Bacc("TRN2", target_bir_lowering=False, debug=False)

    # ---- DRAM I/O (per-core shapes; data differs per core) ----
    d_embT = nc.dram_tensor("embT", [KCH, 128, T], bf, kind="ExternalInput")
    d_headw = nc.dram_tensor("headw", [KCH, 128, HV], bf, kind="ExternalInput")
    d_t1w = nc.dram_tensor("t1w", [K1CH, 128, V1], bf, kind="ExternalInput")
    d_t2w = nc.dram_tensor("t2w", [PK2, V2], bf, kind="ExternalInput")
    d_t1pw = nc.dram_tensor("t1pw", [12, 128, T1_DIM], bf, kind="ExternalInput")
    d_t2pw = nc.dram_tensor("t2pw", [12, 128, T2_DIM], bf, kind="ExternalInput")
    d_embtokT = nc.dram_tensor("embtokT", [KCH, 128, TPC], bf, kind="ExternalInput")
    d_embtok = nc.dram_tensor("embtok", [2, 128, KA], bf, kind="ExternalInput")
    d_ghead = nc.dram_tensor("ghead", [2, 128, KA], bf, kind="ExternalInput")
    d_g1 = nc.dram_tensor("g1", [2, 128, T1_DIM], bf, kind="ExternalInput")
    d_g2 = nc.dram_tensor("g2", [2, 128, T2_DIM], bf, kind="ExternalInput")
    d_sumexp = nc.dram_tensor("sumexp", [3, 128, MCH], f32, kind="ExternalOutput")
    d_gdot = nc.dram_tensor("gdot", [128, 6], f32, kind="ExternalOutput")

    with tile.TileContext(nc) as tc:
        with (
            tc.tile_pool(name="w", bufs=1) as wp,
            tc.tile_pool(name="scr", bufs=4) as scr,
            tc.tile_pool(name="gscr", bufs=1) as gscr,
            tc.tile_pool(name="ps", bufs=4, space="PSUM") as ps,
            tc.tile_pool(name="ps2", bufs=2, space="PSUM") as ps2,
        ):
            # ---- resident loads ----
            embT = wp.tile([128, KCH, T], bf)
            for k in range(KCH):
                nc.sync.dma_start(embT[:, k, :], d_embT[k])
            headw = wp.tile([128, KCH, HV], bf)
            for k in range(KCH):
                nc.sync.dma_start(headw[:, k, :], d_headw[k])
            t1w = wp.tile([128, K1CH, V1], bf)
            for k in range(K1CH):
                nc.sync.dma_start(t1w[:, k, :], d_t1w[k])
            t2w = wp.tile([PK2, V2], bf)
            nc.sync.dma_start(t2w[:], d_t2w[:])
            t1pw = wp.tile([128, 12, T1_DIM], bf)
            for k in range(12):
                nc.sync.dma_start(t1pw[:, k, :], d_t1pw[k])
            t2pw = wp.tile([128, 12, T2_DIM], bf)
            for k in range(12):
                nc.sync.dma_start(t2pw[:, k, :], d_t2pw[k])
            embtokT = wp.tile([128, KCH, TPC], bf)
            for k in range(KCH):
                nc.sync.dma_start(embtokT[:, k, :], d_embtokT[k])
            embtok = wp.tile([128, 2, KA], bf)
            ghead = wp.tile([128, 2, KA], bf)
            for ch in range(2):
                nc.sync.dma_start(embtok[:, ch, :], d_embtok[ch])
                nc.sync.dma_start(ghead[:, ch, :], d_ghead[ch])
            g1 = wp.tile([128, 2, T1_DIM], bf)
            g2 = wp.tile([128, 2, T2_DIM], bf)
            for ch in range(2):
                nc.sync.dma_start(g1[:, ch, :], d_g1[ch])
                nc.sync.dma_start(g2[:, ch, :], d_g2[ch])

            h1T = wp.tile([128, K1CH, T], bf)
            h2T = wp.tile([PK2, T], bf)
            h1tok = wp.tile([128, 2, T1_DIM], bf)
            h2tok = wp.tile([128, 2, T2_DIM], bf)
            sums_h = wp.tile([128, MCH, len(HT)], f32)
            sums_1 = wp.tile([128, MCH, len(T1T)], f32)
            sums_2 = wp.tile([128, MCH, len(T2T)], f32)
            se = wp.tile([128, 3, MCH], f32)
            gd = wp.tile([128, 6], f32)

            nc.vector.memset(se[:], 0.0)
            nc.vector.memset(gd[:], 0.0)
            nc.vector.memset(sums_h[:], 0.0)
            nc.vector.memset(sums_1[:], 0.0)
            nc.vector.memset(sums_2[:], 0.0)

            # ---- phase A: projections (h1T/h2T k-major; h1tok/h2tok token-major) ----
            for mo in range(K1CH if level >= 1 else 0):  # h1T rows 384 -> 3 chunks of 128
                for nt in range(4):  # tokens 2048 -> 4x512
                    pt = ps2.tile([128, 512], mybir.dt.float32)
                    for k in range(12):
                        nc.tensor.matmul(
                            pt,
                            t1pw[:, k, mo * 128:(mo + 1) * 128],
                            embT[:, k, nt * 512:(nt + 1) * 512],
                            start=(k == 0),
                            stop=(k == 11),
                        )
                    nc.scalar.copy(h1T[:, mo, nt * 512:(nt + 1) * 512], pt)
            for nt in range(4 if level >= 1 else 0):
                pt = ps2.tile([PK2, 512], mybir.dt.float32)
                for k in range(12):
                    nc.tensor.matmul(
                        pt,
                        t2pw[:, k, :],
                        embT[:, k, nt * 512:(nt + 1) * 512],
                        start=(k == 0),
                        stop=(k == 11),
                    )
                nc.scalar.copy(h2T[:, nt * 512:(nt + 1) * 512], pt)
            for ch in range(2 if level >= 1 else 0):  # this core's 256 tokens, token-major
                pt = ps2.tile([128, T1_DIM], mybir.dt.float32)
                for k in range(12):
                    nc.tensor.matmul(
                        pt,
                        embtokT[:, k, ch * 128:(ch + 1) * 128],
                        t1pw[:, k, :],
                        start=(k == 0),
                        stop=(k == 11),
                    )
                nc.scalar.copy(h1tok[:, ch, :], pt)
                pt2 = ps2.tile([128, T2_DIM], mybir.dt.float32)
                for k in range(12):
                    nc.tensor.matmul(
                        pt2,
                        embtokT[:, k, ch * 128:(ch + 1) * 128],
                        t2pw[:, k, :],
                        start=(k == 0),
                        stop=(k == 11),
                    )
                nc.scalar.copy(h2tok[:, ch, :], pt2)

            # ---- phases B/C/D: logits + fused exp/row-sum ----
            exp_f = mybir.ActivationFunctionType.Exp

            for m in range(MCH if level >= 2 else 0):
                for j, (off, sz) in enumerate(HT):  # head: K=13 chunks (bias aug)
                    pt = ps.tile([128, 512], mybir.dt.float32, name="lgps")[:, :sz]
                    for k in range(KCH):
                        nc.tensor.matmul(
                            pt,
                            embT[:, k, m * 128:(m + 1) * 128],
                            headw[:, k, off:off + sz],
                            start=(k == 0),
                            stop=(k == KCH - 1),
                        )
                    st = scr.tile([128, 512], mybir.dt.bfloat16, tag="expscr")
                    nc.scalar.activation(st[:, :sz], pt, exp_f)
                    nc.vector.tensor_reduce(
                        sums_h[:, m, j:j + 1], st[:, :sz],
                        axis=mybir.AxisListType.X, op=mybir.AluOpType.add,
                    )
            for m in range(MCH if level >= 3 else 0):
                for j, (off, sz) in enumerate(T1T):
                    pt = ps.tile([128, 512], mybir.dt.float32, name="lgps")[:, :sz]
                    for k in range(K1CH):
                        nc.tensor.matmul(
                            pt,
                            h1T[:, k, m * 128:(m + 1) * 128],
                            t1w[:, k, off:off + sz],
                            start=(k == 0),
                            stop=(k == K1CH - 1),
                        )
                    st = scr.tile([128, 512], mybir.dt.bfloat16, tag="expscr")
                    nc.scalar.activation(st[:, :sz], pt, exp_f)
                    nc.vector.tensor_reduce(
                        sums_1[:, m, j:j + 1], st[:, :sz],
                        axis=mybir.AxisListType.X, op=mybir.AluOpType.add,
                    )
            for m in range(MCH if level >= 4 else 0):
                for j, (off, sz) in enumerate(T2T):
                    pt = ps.tile([128, 512], mybir.dt.float32, name="lgps")[:, :sz]
                    nc.tensor.matmul(
                        pt,
                        h2T[:, m * 128:(m + 1) * 128],
                        t2w[:, off:off + sz],
                        start=True,
                        stop=True,
                    )
                    st = scr.tile([128, 512], mybir.dt.bfloat16, tag="expscr")
                    nc.scalar.activation(st[:, :sz], pt, exp_f)
                    nc.vector.tensor_reduce(
                        sums_2[:, m, j:j + 1], st[:, :sz],
                        axis=mybir.AxisListType.X, op=mybir.AluOpType.add,
                    )

            # ---- phase E: gathered target logits (row-wise dots, 256 tok/core) ----
            mult = mybir.AluOpType.mult
            add = mybir.AluOpType.add
            for ch in range(2 if level >= 5 else 0):
                so = gscr.tile([128, KA], mybir.dt.float32, tag="ttr_scr")
                nc.vector.tensor_mul(so[:], embtok[:, ch, :], ghead[:, ch, :])
                nc.vector.tensor_reduce(
                    gd[:, 0 + ch:1 + ch], so[:],
                    axis=mybir.AxisListType.X, op=add,
                )
                so1 = gscr.tile([128, T1_DIM], mybir.dt.float32, tag="ttr_scr1")
                nc.vector.tensor_mul(so1[:], h1tok[:, ch, :], g1[:, ch, :])
                nc.vector.tensor_reduce(
                    gd[:, 2 + ch:3 + ch], so1[:],
                    axis=mybir.AxisListType.X, op=add,
                )
                so2 = gscr.tile([128, T2_DIM], mybir.dt.float32, tag="ttr_scr2")
                nc.vector.tensor_mul(so2[:], h2tok[:, ch, :], g2[:, ch, :])
                nc.vector.tensor_reduce(
                    gd[:, 4 + ch:5 + ch], so2[:],
                    axis=mybir.AxisListType.X, op=add,
                )

            # ---- phase F: fold per-tile partials, write out ----
            nc.vector.tensor_reduce(
                se[:, 0, :], sums_h[:], axis=mybir.AxisListType.X, op=add
            )
            nc.vector.tensor_reduce(
                se[:, 1, :], sums_1[:], axis=mybir.AxisListType.X, op=add
            )
            nc.vector.tensor_reduce(
                se[:, 2, :], sums_2[:], axis=mybir.AxisListType.X, op=add
            )
            for c in range(3):
                nc.sync.dma_start(d_sumexp[c], se[:, c, :])
            nc.sync.dma_start(d_gdot[:], gd[:])

    nc.finalize()
    _CACHE[("nc", level)] = nc
    return nc


def _prep_inputs(targets, last_hidden_states, cls_tokens_batch, example_ids,
                 token_ids, head_w, head_b, t1_proj_w, t1_out_w, t2_proj_w,
                 t2_out_w):
    """Host-side shard prep. Returns (in_maps, masks, gather indices)."""
    f32 = np.float32
    targets = np.asarray(targets).astype(np.int64)
    example_ids = np.asarray(example_ids).astype(np.int64)
    token_ids = np.asarray(token_ids).astype(np.int64)
    lhs = np.asarray(last_hidden_states, dtype=f32)
    cls = np.asarray(cls_tokens_batch, dtype=f32)
    head_w = np.asarray(head_w, dtype=f32)
    head_b = np.asarray(head_b, dtype=f32)
    t1_proj_w = np.asarray(t1_proj_w, dtype=f32)
    t1_out_w = np.asarray(t1_out_w, dtype=f32)
    t2_proj_w = np.asarray(t2_proj_w, dtype=f32)
    t2_out_w = np.asarray(t2_out_w, dtype=f32)

    # gather + concat + bias-augment: emb_aug [T, KA]
    emb_aug = np.zeros((T, KA), dtype=f32)
    emb_aug[:, :HDIM] = cls[example_ids]
    emb_aug[:, HDIM:IN_F] = lhs[example_ids, token_ids]
    emb_aug[:, IN_F] = 1.0

    in_c1 = (targets >= CUTOFFS[0]) & (targets < CUTOFFS[1])
    in_c2 = targets >= CUTOFFS[1]
    gihead = np.where(in_c1, SHORTLIST, np.where(in_c2, SHORTLIST + 1, targets))
    rel1 = np.clip(targets - CUTOFFS[0], 0, T1_SIZE - 1)
    rel2 = np.clip(targets - CUTOFFS[1], 0, T2_SIZE - 1)

    embT_b = np.ascontiguousarray(emb_aug.T).astype(BF16).reshape(KCH, 128, T)
    emb_bf = emb_aug.astype(BF16)                    # token-major [T, KA]

    # head: augmented + transposed + vocab-padded [KA, HVPAD]
    hw = np.zeros((KA, HVPAD), dtype=f32)
    hw[:IN_F, :HEAD_SIZE] = head_w.T
    hw[IN_F, :HEAD_SIZE] = head_b
    hw[IN_F, HEAD_SIZE:] = PAD_BIAS
    hw = hw.astype(BF16).reshape(KCH, 128, HVPAD)

    t1wT = np.ascontiguousarray(t1_out_w.T).astype(BF16).reshape(K1CH, 128, T1_SIZE)
    t2wT = np.ascontiguousarray(t2_out_w.T).astype(BF16)          # [96, 60000]
    t1pwT = np.ascontiguousarray(t1_proj_w.T).astype(BF16).reshape(12, 128, T1_DIM)
    t2pwT = np.ascontiguousarray(t2_proj_w.T).astype(BF16).reshape(12, 128, T2_DIM)

    # gathered weight rows (target logit = row-dot), bias folded into col IN_F
    gh = np.zeros((T, KA), dtype=f32)
    gh[:, :IN_F] = head_w[gihead]
    gh[:, IN_F] = head_b[gihead]
    gh = gh.astype(BF16)
    g1r = t1_out_w[rel1].astype(BF16)
    g2r = t2_out_w[rel2].astype(BF16)

    in_maps = []
    for c in range(NCORES):
        ts_ = slice(c * TPC, (c + 1) * TPC)
        in_maps.append({
            "embT": embT_b,
            "headw": np.ascontiguousarray(hw[:, :, c * HV:(c + 1) * HV]),
            "t1w": np.ascontiguousarray(t1wT[:, :, c * V1:(c + 1) * V1]),
            "t2w": np.ascontiguousarray(t2wT[:, c * V2:(c + 1) * V2]),
            "t1pw": t1pwT,
            "t2pw": t2pwT,
            "embtokT": np.ascontiguousarray(embT_b[:, :, c * TPC:(c + 1) * TPC]),
            "embtok": np.ascontiguousarray(emb_bf[ts_]).reshape(2, 128, KA),
            "ghead": np.ascontiguousarray(gh[ts_]).reshape(2, 128, KA),
            "g1": np.ascontiguousarray(g1r[ts_]).reshape(2, 128, T1_DIM),
            "g2": np.ascontiguousarray(g2r[ts_]).reshape(2, 128, T2_DIM),
        })
    return in_maps, in_c1, in_c2


def _combine(results, in_c1, in_c2):
    """Host-side unshard: sum sumexp partials, assemble gathered logits."""
    sumexp = np.zeros((3, T), dtype=np.float64)
    g = np.zeros((3, T), dtype=np.float64)
    for c, res in enumerate(results):
        se = np.asarray(res["sumexp"], dtype=np.float64)   # [3, 128, MCH]
        # token t = m*128 + p  ->  se[:, p, m]
        sumexp += se.transpose(0, 2, 1).reshape(3, T)
        gd = np.asarray(res["gdot"], dtype=np.float64)     # [128, 6]
        for cl in range(3):
            for ch in range(2):
                t0 = c * TPC + ch * 128
                g[cl, t0:t0 + 128] = gd[:, cl * 2 + ch]
    lse = np.log(sumexp)
    out = g[0] - lse[0]
    out = out + np.where(in_c1, g[1] - lse[1], 0.0)
    out = out + np.where(in_c2, g[2] - lse[2], 0.0)
    loss = -out.mean()
    return out.astype(np.float32), np.float32(loss)


def kernel(**inputs):
    nc = _build_program()
    in_maps, in_c1, in_c2 = _prep_inputs(**inputs)
    res = run_bass_kernel_spmd(nc, in_maps, core_ids=list(range(NCORES)))
    return _combine(res.results, in_c1, in_c2)
